# revision 59
# baseline (speedup 1.0000x reference)
"""Trainium2 Bass kernel for nn_DSFE (dual attention + LFE depthwise-conv block).

Sharding: pure data-parallel over batch B=16 across 8 NeuronCores (2 batches
per core), no collectives.

v2 design (vs v0 baseline):
  * bf16 everywhere on the PE (1 cycle/row at any width, 1024-wide moving
    operands, 1c/row transposes); PSUM accumulation stays fp32.  Empirical
    bf16-everywhere error vs the fp32 reference: 3.6e-3 absmax-rel (budget
    2e-2).  fp8 measured 3-6e-2 -> rejected.
  * spatial index n' = 32w + h (x's native memory order) everywhere: x loads,
    XC transposes, y stores are all contiguous; E/F weights are loaded through
    a (h w)->(w h) permuted AP; the one place the torch reshape semantically
    mixes spatial order into channels (x_sa scramble) reads at_sb through an
    n_ref-permuted AP when forming x_sa lhsT chunks.
  * all transposed weights (qkvvT/fc1T/fc2T/owt/owt2/ewt/fwt) are cast to
    bf16 and kept resident in SBUF - no DRAM scratch roundtrip, no per-batch
    weight reloads.
  * 3x3 depthwise conv: 9 shifted-AP taps per oc-tile accumulated in a bf16
    SBUF accumulator - center+5 on DVE (2x 16-bit mode), 2 on GPSIMD, h-wrap
    column fixups on GPSIMD; conv is entirely off the PE.
  * PSUM evictions split between ACT (activation Identity, with fused
    bias/scale/cast) and DVE to balance engine load.
"""

from contextlib import ExitStack

import numpy as np

import concourse.bass as bass
import concourse.mybir as mybir
import concourse.tile as tile
from concourse.masks import make_identity

FP = mybir.dt.float32
BF = mybir.dt.bfloat16
AF = mybir.ActivationFunctionType
ALU = mybir.AluOpType
ts = bass.ts

B, W_, H_, C = 16, 32, 32, 512
HEADS, PP, HID = 4, 16, 2048
N = H_ * W_            # 1024
D = C // HEADS         # 128
NCORES = 8
BL = B // NCORES       # 2 batches per core
NCH = N // 128         # 8
CCH = C // 128         # 4
OCH = HID // 128       # 16

NORM_EPS = 1e-12

# depthwise 3x3 taps (kh, kw); spatial n' = 32w + h so shift = oh + 32*ow.
# PE_TAPS run as bf16 diag matmuls accumulated in PSUM; the first DVE op
# merges the PSUM partial with the center tap (stt in1=psum); remaining taps
# split DVE/GPSIMD.  oh != 0 taps need an h-wrap column fixup (split DVE/GPS).
CENTER_TAP = (1, 1)
PE_TAPS = [(0, 0), (0, 2), (2, 0), (2, 2), (1, 2), (2, 1)]
DVE_TAPS = [(0, 1), (1, 0)]
GPS_TAPS = []
DVE_FIX = [(0, 0), (0, 1), (0, 2), (2, 2), (2, 0), (2, 1)]
GPS_FIX = []


def split_multi_waits(nc):
    """This environment's walrus build encodes at most ONE sync wait per
    instruction; re-host excess waits onto same-engine NoOps just before."""
    n_split = 0
    for f in nc.m.functions:
        for blk in f.blocks:
            new = []
            changed = False
            for inst in blk.instructions:
                si = inst.sync_info
                waits = list(si.on_wait) if si and si.on_wait else []
                if len(waits) > 1:
                    n_split += 1
                    changed = True
                    for w in waits[:-1]:
                        new.append(mybir.InstNoOp(
                            name=nc.get_next_instruction_name(),
                            sync_info=mybir.SyncInfo(on_wait=[w], on_update=[]),
                            bass_nofuse=True,
                            engine=inst.engine,
                        ))
                    inst.sync_info = mybir.SyncInfo(
                        on_wait=[waits[-1]],
                        on_update=list(si.on_update) if si.on_update else [],
                    )
                new.append(inst)
            if changed:
                blk.instructions = new
    return n_split


def build(split_waits=True, sim_gelu=False, loop=1, phases="AB"):
    """Build the per-core Bass module (SPMD: identical program on 8 cores)."""
    nc = bass.Bass("TRN2", target_bir_lowering=False, debug=False)

    def din(name, shape):
        return nc.dram_tensor(name, list(shape), FP, kind="ExternalInput").ap()

    aps = dict(
        x=din("x", (BL, W_, H_, C)),
        qkvv_w=din("qkvv_w", (4 * C, C)),
        e_w=din("E_w", (PP, N)),
        e_b=din("E_b", (PP,)),
        f_w=din("F_w", (PP, N)),
        f_b=din("F_b", (PP,)),
        temp=din("temp", (HEADS, 1, 1)),
        temp2=din("temp2", (HEADS, 1, 1)),
        out_w=din("out_w", (C // 2, C)),
        out_b=din("out_b", (C // 2,)),
        out2_w=din("out2_w", (C // 2, C)),
        out2_b=din("out2_b", (C // 2,)),
        fc1_w=din("fc1_w", (HID, C)),
        fc1_b=din("fc1_b", (HID,)),
        dw_w=din("dw_w", (HID, 1, 3, 3)),
        dw_b=din("dw_b", (HID,)),
        fc2_w=din("fc2_w", (C, HID)),
        fc2_b=din("fc2_b", (C,)),
        y=nc.dram_tensor("y", [BL, W_, H_, C], FP, kind="ExternalOutput").ap(),
    )

    with tile.TileContext(nc) as tc:
        _emit(nc, tc, aps, sim_gelu, loop, phases)

    if split_waits:
        split_multi_waits(nc)
    return nc


def _transpose_weight_bf(nc, tc, pools, identb, nat_ap, dst, label):
    """Transpose a natural (R, K) fp32 DRAM weight into K-major bf16 SBUF
    tiles dst[K//128] of shape (128, R).

    Casting gpsimd DMAs load fp32 DRAM directly into bf16 SBUF (one per
    8-row-chunk group, so transposes start after the first chunk lands);
    bf16 PE transposes batched 8 row-chunks per (128,1024) PSUM tile, one
    DVE evict each."""
    act = nc.scalar
    dve = nc.vector
    pe = nc.tensor
    R, K = nat_ap.shape
    RC, KC = R // 128, K // 128
    G = min(8, RC)  # row-chunks per PSUM eviction batch
    with ExitStack() as es:
        nat_pool = es.enter_context(tc.tile_pool(name=f"twn_{label}", bufs=1))
        nat = nat_pool.tile([128, RC, K], FP, tag="nat", name=f"nat_{label}")
        natb = nat_pool.tile([128, RC, K], BF, tag="natb", name=f"natb_{label}")
        natsrc = nat_ap.rearrange("(g p) k -> p g k", p=128)
        for g in range((RC + G - 1) // G):
            g0, g1 = g * G, min((g + 1) * G, RC)
            nc.sync.dma_start(nat[:, g0:g1, :], natsrc[:, g0:g1, :])
        for rj in range(RC):
            act.activation(natb[:, rj, :], nat[:, rj, :], AF.Identity)
        for g in range((RC + G - 1) // G):
            rjs = list(range(g * G, min((g + 1) * G, RC)))
            for ci in range(KC):
                ps = pools["tr"].tile([128, 1024], BF, tag="tr",
                                      name=f"tw_{label}{g}_{ci}")
                for j, rj in enumerate(rjs):
                    pe.transpose(ps[:, ts(j, 128)], natb[:, rj, ts(ci, 128)],
                                 identb[:])
                dve.tensor_copy(
                    dst[ci][:, g * G * 128:g * G * 128 + len(rjs) * 128],
                    ps[:, 0:len(rjs) * 128])


def _emit(nc, tc, aps, sim_gelu=False, loop=1, phases="AB"):
    act = nc.scalar
    dve = nc.vector
    gps = nc.gpsimd
    pe = nc.tensor

    with ExitStack() as es:
        const = es.enter_context(tc.tile_pool(name="const", bufs=1))

        # one shared set of PSUM pools for the whole program (8 banks total):
        # scoped per-phase pools would serialize phase/batch overlap through
        # bank reuse barriers.
        ps_big = es.enter_context(tc.tile_pool(name="ps_big", bufs=2, space="PSUM"))
        ps_tr = es.enter_context(tc.tile_pool(name="ps_tr", bufs=2, space="PSUM"))
        ps_sm = es.enter_context(tc.tile_pool(name="ps_sm", bufs=2, space="PSUM"))
        pools = dict(big=ps_big, tr=ps_tr, sm=ps_sm)

        identb = const.tile([128, 128], BF)
        make_identity(nc, identb)
        ident32 = const.tile([128, 128], FP)
        make_identity(nc, ident32)
        ones_row = const.tile([1, 128], BF)
        dve.memset(ones_row[:], 1.0)
        ones_col = const.tile([128, 1], BF)
        dve.memset(ones_col[:], 1.0)

        # resident bf16 transposed weights
        qkvvT = [const.tile([128, 4 * C], BF, tag=f"qkvvT{i}", name=f"qkvvT{i}") for i in range(CCH)]
        _transpose_weight_bf(nc, tc, pools, identb, aps["qkvv_w"], qkvvT, "qkvv")
        fc1T = [const.tile([128, HID], BF, tag=f"fc1T{i}", name=f"fc1T{i}") for i in range(CCH)]
        _transpose_weight_bf(nc, tc, pools, identb, aps["fc1_w"], fc1T, "fc1")
        fc2T = [const.tile([128, C], BF, tag=f"fc2T{o}", name=f"fc2T{o}") for o in range(OCH)]
        _transpose_weight_bf(nc, tc, pools, identb, aps["fc2_w"], fc2T, "fc2")
        owt = [const.tile([128, C // 2], BF, tag=f"owt{q}", name=f"owt{q}") for q in range(CCH)]
        _transpose_weight_bf(nc, tc, pools, identb, aps["out_w"], owt, "outw")
        owt2 = [const.tile([128, C // 2], BF, tag=f"owt2{q}", name=f"owt2{q}") for q in range(CCH)]
        _transpose_weight_bf(nc, tc, pools, identb, aps["out2_w"], owt2, "outw2")

        # temp/temp2 broadcast to all 128 partitions: (128, HEADS) fp32
        tcol = const.tile([128, HEADS], FP)
        nc.sync.dma_start(
            tcol[:], aps["temp"].rearrange("h a b -> (a b) h").to_broadcast((128, HEADS)))
        t2col = const.tile([128, HEADS], FP)
        nc.sync.dma_start(
            t2col[:], aps["temp2"].rearrange("h a b -> (a b) h").to_broadcast((128, HEADS)))

        # E^T / F^T as (n'-part r, n'-chunk k, p) bf16; n' order via (h w)->(w h)
        ewt = const.tile([128, NCH, PP], BF)
        fwt = const.tile([128, NCH, PP], BF)
        with ExitStack() as ef:
            ef_pool = ef.enter_context(tc.tile_pool(name="ef_nat", bufs=2))
            for nm, dst in (("e_w", ewt), ("f_w", fwt)):
                natf = ef_pool.tile([PP, N], FP, tag="natf", name=f"{nm}_natf")
                nc.sync.dma_start(natf[:], aps[nm][:])
                natb = ef_pool.tile([PP, N], BF, tag="natb", name=f"{nm}_natb")
                act.activation(natb[:], natf[:], AF.Identity)
                # reorder columns to n' = 32w + h with a permuted DVE copy
                # (matmul operands must have a single free dimension)
                nprm = ef_pool.tile([PP, N], BF, tag="nprm", name=f"{nm}_nprm")
                dve.tensor_copy(nprm[:].rearrange("p (w h) -> p w h", h=H_),
                                natb[:].rearrange("p (h w) -> p w h", w=W_))
                for k in range(NCH):
                    ps = pools["sm"].tile([128, 512], FP, tag="sm",
                                          name=f"efps_{nm}{k}")
                    psv = ps[:, 0:PP // 2].bitcast(BF)
                    pe.transpose(psv, nprm[:, ts(k, 128)], identb[0:PP, 0:PP])
                    dve.tensor_copy(dst[:, k, :], psv)
        eb_row = const.tile([1, PP], BF)
        e_b32 = const.tile([1, PP], FP)
        nc.sync.dma_start(e_b32[:], aps["e_b"].unsqueeze(0))
        dve.tensor_copy(eb_row[:], e_b32[:])
        fb_row = const.tile([1, PP], BF)
        f_b32 = const.tile([1, PP], FP)
        nc.sync.dma_start(f_b32[:], aps["f_b"].unsqueeze(0))
        dve.tensor_copy(fb_row[:], f_b32[:])

        # free-dim bias row for OUTA: concat(out_b, out2_b) + fc2_b, bf16
        br_cat = const.tile([1, C], FP)
        nc.sync.dma_start(br_cat[:, 0:C // 2], aps["out_b"].unsqueeze(0))
        nc.sync.dma_start(br_cat[:, C // 2:C], aps["out2_b"].unsqueeze(0))
        fcb_row = const.tile([1, C], FP)
        nc.sync.dma_start(fcb_row[:], aps["fc2_b"].unsqueeze(0))
        brf = const.tile([1, C], BF)
        dve.tensor_add(brf[:], br_cat[:], fcb_row[:])

        # per-partition (o) bias/weight columns for the LFE branch (fp32)
        fb1 = const.tile([128, OCH], FP)
        nc.sync.dma_start(fb1[:], aps["fc1_b"].rearrange("(k r) -> r k", r=128))
        dwb = const.tile([128, OCH], FP)
        nc.sync.dma_start(dwb[:], aps["dw_b"].rearrange("(k r) -> r k", r=128))
        dww = const.tile([128, OCH * 9], FP)
        for oc in range(OCH):
            nc.sync.dma_start(
                dww[:, oc * 9:(oc + 1) * 9],
                aps["dw_w"][oc * 128:(oc + 1) * 128].rearrange("r a kh kw -> r (a kh kw)"))
        ndww = const.tile([128, OCH * 9], FP)
        dve.tensor_scalar_mul(ndww[:], dww[:], -1.0)

        # bf16 diag(w_tap) tiles for the PE conv taps: (128, OCH, |PE_TAPS|, 128)
        NPT = len(PE_TAPS)
        dgP = const.tile([128, OCH * NPT * 128], BF)
        dg4 = dgP[:].rearrange("p (o t j) -> p o t j", t=NPT, j=128)
        for oc in range(OCH):
            wsel = const.tile([128, NPT], FP, tag="wsel", name=f"wsel{oc}")
            for t, (kh, kw) in enumerate(PE_TAPS):
                dve.tensor_copy(wsel[:, t:t + 1],
                                dww[:, oc * 9 + 3 * kh + kw:oc * 9 + 3 * kh + kw + 1])
            dve.tensor_mul(dg4[:, oc, :, :],
                           identb[:].unsqueeze(1).to_broadcast((128, NPT, 128)),
                           wsel[:].unsqueeze(2).to_broadcast((128, NPT, 128)))

        cst = dict(identb=identb, ident32=ident32, ones_row=ones_row,
                   ones_col=ones_col,
                   ndww=ndww, dgP=dgP, tcol=tcol, t2col=t2col,
                   ewt=ewt, fwt=fwt, eb_row=eb_row, fb_row=fb_row, brf=brf,
                   fb1=fb1, dwb=dwb, dww=dww, owt=owt, owt2=owt2,
                   qkvvT=qkvvT, fc1T=fc1T, fc2T=fc2T)

        for rep in range(loop):
            for b in range(BL):
                _emit_batch(nc, tc, b, aps, cst, pools, sim_gelu,
                            label=f"{rep}_{b}", phases=phases)


def _emit_batch(nc, tc, b, aps, cst, pools, sim_gelu=False, label=None, phases="AB"):
    if label is None:
        label = str(b)
    act = nc.scalar
    dve = nc.vector
    gps = nc.gpsimd
    pe = nc.tensor
    identb = cst["identb"]
    ones_row = cst["ones_row"]

    with ExitStack() as bs:
        xc_pool = bs.enter_context(tc.tile_pool(name=f"xc{label}", bufs=1))
        outa_pool = bs.enter_context(tc.tile_pool(name=f"outa{label}", bufs=1))

        # ---- XC (c, n') bf16, n' = 32w + h == x's native row order.
        xcb = [xc_pool.tile([128, N], BF, tag=f"xc{i}", name=f"xc{i}")
               for i in range(CCH)]
        with ExitStack() as xl:
            xn_pool = xl.enter_context(tc.tile_pool(name=f"xn{label}", bufs=1))
            xn = xn_pool.tile([128, NCH, C], FP, tag="xn", name=f"xn{label}")
            xnb = xn_pool.tile([128, NCH, C], BF, tag="xnb", name=f"xnb{label}")
            xsrc = (aps["x"][b].rearrange("w h c -> (w h) c")
                    .rearrange("(g p) c -> p g c", p=128))
            for g in range(2):
                nc.sync.dma_start(xn[:, 4 * g:4 * (g + 1), :],
                                  xsrc[:, 4 * g:4 * (g + 1), :])
            for kk in range(NCH):
                act.activation(xnb[:, kk, :], xn[:, kk, :], AF.Identity)
            for i in range(CCH):
                ps = pools["tr"].tile([128, 1024], BF, tag="tr",
                                      name=f"xct{i}")
                for kk in range(NCH):
                    pe.transpose(ps[:, ts(kk, 128)], xnb[:, kk, ts(i, 128)],
                                 identb[:])
                dve.tensor_copy(xcb[i][:], ps[:])

        outa = [outa_pool.tile([128, C], FP, tag=f"outa{m}", name=f"outa{m}") for m in range(NCH)]
        if phases != "AB":
            for m in range(NCH):
                dve.memset(outa[m][:], 0.0)
        if "B" not in phases:
            ydst0 = aps["y"][b].rearrange("w h c -> (w h) c")
            nc.sync.dma_start(ydst0[0:128], outa[0][:])

        # ======================= phase A: attention =======================
        if "A" in phases:
          with ExitStack() as pa:
            qkvv_pool = pa.enter_context(tc.tile_pool(name=f"qkvv{label}", bufs=1))
            ps_big = pools["big"]
            ps_t = pools["tr"]
            # one (128,512) fp32 bank per head rotation hosts every small
            # PSUM tensor as a slot view (PSUM tiles round up to full banks)
            ps_med = pools["sm"]

            # ---- QKVV (n', 4C) bf16, plus elementwise squares of the q|k
            # half for the hoisted norm computation
            qkvv = [qkvv_pool.tile([128, 4 * C], BF, tag=f"qkvv{k}", name=f"qkvv{k}") for k in range(NCH)]
            sqq = qkvv_pool.tile([128, NCH, 1024], BF, tag="sqq", name=f"sqq{label}")
            for k in range(NCH):
                for half in range(2):
                    ps = ps_big.tile([128, 1024], FP, tag="big", name=f"qkps{k}_{half}")
                    for j2 in range(2):
                        for i in range(CCH):
                            pe.matmul(ps[:, ts(j2, 512)], xcb[i][:, ts(k, 128)],
                                      cst["qkvvT"][i][:, half * 1024 + j2 * 512:half * 1024 + (j2 + 1) * 512],
                                      start=(i == 0), stop=(i == CCH - 1))
                    act.activation(qkvv[k][:, ts(half, 1024)], ps[:], AF.Identity)
                act.activation(sqq[:, k, :], qkvv[k][:, 0:1024], AF.Square)

            with ExitStack() as ph:
                at_pool = pa.enter_context(tc.tile_pool(name=f"at{label}", bufs=2))
                hd_pool = pa.enter_context(tc.tile_pool(name=f"hd{label}", bufs=1))
                xca_pool = pa.enter_context(tc.tile_pool(name=f"xca{label}", bufs=1))
                xsa_pool = pa.enter_context(tc.tile_pool(name=f"xsa{label}", bufs=1))

                xsa = [xsa_pool.tile([128, N], BF, tag=f"xsa{q}", name=f"xsa{q}") for q in range(CCH)]
                xca = []
                at_sbs = []
                vpt_sbs = []

                # ---- hoisted norms for all heads: ssq[d, c8] over n' via
                # ones-matmuls (c8 = 4 q-heads then 4 k-heads), then one
                # sqrt/recip, and the per-head CA column scale as one
                # (128, 4*128) outer product.  norms are O(sqrt(N)) here, so
                # the reference's 1e-12 clamp is a no-op.
                hsm = ps_med.tile([128, 512], FP, tag="sm", name=f"hsm{label}")
                nrm_ps = hsm[:, 0:8]
                for c8 in range(8):
                    for k in range(NCH):
                        pe.matmul(nrm_ps[:, c8:c8 + 1],
                                  sqq[:, k, ts(c8, 128)], cst["ones_col"][:],
                                  start=(k == 0), stop=(k == NCH - 1),
                                  skip_group_check=True)
                nrm8 = at_pool.tile([128, 8], FP, tag="nrm8")
                act.activation(nrm8[:], nrm_ps, AF.Sqrt)
                inv8 = at_pool.tile([128, 8], FP, tag="inv8")
                dve.reciprocal(inv8[:], nrm8[:])
                invq_t4 = at_pool.tile([128, HEADS], FP, tag="invq_t4")
                dve.tensor_mul(invq_t4[:], inv8[:, 0:4], cst["tcol"][:])
                invkb4 = at_pool.tile([128, HEADS], BF, tag="invkb4")
                dve.tensor_copy(invkb4[:], inv8[:, 4:8])
                # per-head (128,1)->(1,128) transposes, all on partition 0
                # (partition-offset reads must be 32-aligned in walrus)
                ikr_row = at_pool.tile([1, 4 * 128], BF, tag="ikr_row")
                for h in range(HEADS):
                    ikr_ps_h = hsm[0:1, 16 + 64 * h:80 + 64 * h].bitcast(BF)
                    pe.transpose(ikr_ps_h, invkb4[:, h:h + 1], identb[:])
                    dve.tensor_copy(ikr_row[:, ts(h, 128)], ikr_ps_h)
                csm = ps_med.tile([128, 512], FP, tag="sm", name=f"csm{label}")
                pe.matmul(csm[:], ones_row[:], ikr_row[:], start=True, stop=True)
                colsc_sb = at_pool.tile([128, 512], BF, tag="colsc_sb")
                dve.tensor_copy(colsc_sb[:], csm[:])

                for h in range(HEADS):
                    qc = h * 128            # q columns in QKVV
                    kc = C + h * 128        # k columns
                    vc = 2 * C + h * 128    # v_ca columns
                    sc = 3 * C + h * 128    # v_sa columns

                    # one bank hosts all small PSUM tensors for this head:
                    #   [0:128) s_ps / (reused) [0:64) et_ps(bf16)
                    #   [128:256) a_ps   [256:272) kp   [272:288) vp
                    #   [288:352) p0:16 vpt(bf16)
                    sm = ps_med.tile([128, 512], FP, tag="sm", name=f"sm{h}")
                    s_ps = sm[:, 0:128]
                    et_ps = sm[:, 0:64].bitcast(BF)
                    a_ps = sm[:, 128:256]
                    kp_ps = sm[:, 256:272]
                    vp_ps = sm[:, 272:288]
                    vpt_ps = sm[0:16, 288:352].bitcast(BF)

                    # ---- q^T (d, n') -> qn_t
                    qT_ps = ps_t.tile([128, N], BF, tag="tr", name=f"qT{h}")
                    for k in range(NCH):
                        pe.transpose(qT_ps[:, ts(k, 128)],
                                     qkvv[k][:, qc:qc + 128], identb[:])
                    qn_t = at_pool.tile([128, N], BF, tag="qn_t")
                    act.activation(qn_t[:], qT_ps[:], AF.Identity,
                                   scale=inv8[:, h:h + 1])

                    # ---- CA scores S0 = q @ k^T (d x d)
                    for k in range(NCH):
                        pe.matmul(s_ps, qkvv[k][:, qc:qc + 128],
                                  qkvv[k][:, kc:kc + 128],
                                  start=(k == 0), stop=(k == NCH - 1))
                    s_sb = at_pool.tile([128, 128], FP, tag="s_sb")
                    dve.scalar_tensor_tensor(s_sb[:], s_ps, invq_t4[:, h:h + 1],
                                             colsc_sb[:, ts(h, 128)],
                                             op0=ALU.mult, op1=ALU.mult)

                    # ---- CA row softmax, no max-subtraction: logits are
                    # temp * cos-similarity, |logit| <= |temp| (bounded) so
                    # exp cannot overflow.  1/sum folded into the x_ca evict.
                    e_sb = at_pool.tile([128, 128], BF, tag="e_sb")
                    sume = at_pool.tile([128, 1], FP, tag="sume")
                    act.activation(e_sb[:], s_sb[:], AF.Exp, accum_out=sume[:])
                    rex = at_pool.tile([128, 1], FP, tag="rex")
                    dve.reciprocal(rex[:], sume[:])

                    pe.transpose(et_ps, e_sb[:], identb[:])
                    et_sb = at_pool.tile([128, 128], BF, tag="et_sb")
                    dve.tensor_copy(et_sb[:], et_ps)

                    # ---- v_ca^T (e, n')
                    vt_ps = ps_t.tile([128, N], BF, tag="tr", name=f"vT{h}")
                    for k in range(NCH):
                        pe.transpose(vt_ps[:, ts(k, 128)],
                                     qkvv[k][:, vc:vc + 128], identb[:])
                    vt_sb = at_pool.tile([128, N], BF, tag="vt_sb")
                    act.activation(vt_sb[:], vt_ps[:], AF.Identity)

                    # ---- x_ca (d, n') = (1/sum) * exp(S)^T.T @ v_ca^T
                    xca_ps = ps_big.tile([128, N], FP, tag="big", name=f"xcaps{h}")
                    for j2 in range(2):
                        pe.matmul(xca_ps[:, ts(j2, 512)], et_sb[:],
                                  vt_sb[:, ts(j2, 512)], start=True, stop=True)
                    xca_h = xca_pool.tile([128, N], BF, tag=f"xca{h}")
                    act.activation(xca_h[:], xca_ps[:], AF.Identity, scale=rex[:])
                    xca.append(xca_h)

                    # ---- SA: k_proj / v_proj (d, p), bias first
                    pe.matmul(kp_ps, ones_row[:], cst["eb_row"][:],
                              start=True, stop=False)
                    for k in range(NCH):
                        pe.matmul(kp_ps, qkvv[k][:, kc:kc + 128],
                                  cst["ewt"][:, k, :], start=False, stop=(k == NCH - 1))
                    kp_sb = at_pool.tile([128, PP], BF, tag="kp_sb")
                    dve.tensor_copy(kp_sb[:], kp_ps)

                    pe.matmul(vp_ps, ones_row[:], cst["fb_row"][:],
                              start=True, stop=False)
                    for k in range(NCH):
                        pe.matmul(vp_ps, qkvv[k][:, sc:sc + 128],
                                  cst["fwt"][:, k, :], start=False, stop=(k == NCH - 1))
                    vp_sb = at_pool.tile([128, PP], BF, tag="vp_sb")
                    dve.tensor_copy(vp_sb[:], vp_ps)
                    pe.transpose(vpt_ps, vp_sb[:], identb[:])
                    vpt_sb = hd_pool.tile([16, 128], BF, tag=f"vpt{h}", name=f"vpt{h}")
                    dve.tensor_copy(vpt_sb[:], vpt_ps)
                    vpt_sbs.append(vpt_sb)

                    # ---- A0 (n', p) per n'-chunk, all 8 in one (128, 8, 16)
                    a3 = a_ps.rearrange("p (k s) -> p k s", s=PP)
                    for k in range(NCH):
                        pe.matmul(a3[:, k, :], qn_t[:, ts(k, 128)],
                                  kp_sb[:], start=True, stop=True)

                    # ---- segmented softmax over p, no max-subtraction:
                    # logits = temp2 * (unit qn row) . k_proj columns are
                    # bounded well inside exp range for this problem's scales
                    ez = at_pool.tile([128, 128], FP, tag="ez")
                    act.activation(ez[:], a_ps, AF.Exp, scale=cst["t2col"][:, h:h + 1])
                    ez3 = ez[:].rearrange("p (k s) -> p k s", s=PP)
                    esum = at_pool.tile([128, NCH], FP, tag="esum")
                    dve.tensor_reduce(esum[:], ez3, axis=mybir.AxisListType.X,
                                      op=ALU.add)
                    rsum = at_pool.tile([128, NCH], FP, tag="rsum")
                    dve.reciprocal(rsum[:], esum[:])
                    attn = at_pool.tile([128, 128], BF, tag="attn")
                    attn3 = attn[:].rearrange("p (k s) -> p k s", s=PP)
                    dve.tensor_mul(attn3, ez3,
                                   rsum[:].unsqueeze(2).to_broadcast((128, NCH, PP)))

                    # ---- attn^T (p, n')
                    at_full = ps_t.tile([128, N], BF, tag="tr", name=f"atps{h}")
                    for k in range(NCH):
                        pe.transpose(at_full[0:16, ts(k, 128)], attn3[:, k, :],
                                     identb[:])
                    # evict directly in n_ref order (n' -> n_ref permuted
                    # read) so x_sa lhsT chunks are flat single-free-dim APs
                    at_sb = hd_pool.tile([16, N], BF, tag=f"at{h}")
                    dve.tensor_copy(
                        at_sb[:].rearrange("p (h2 w) -> p h2 w", w=W_),
                        at_full[0:16, :].rearrange("p (w h2) -> p h2 w", w=W_))
                    at_sbs.append(at_sb)

                # ---- x_sa in the torch-scrambled layout: at_sb is already
                # in n_ref column order, so lhsT chunks are plain slices;
                # xsa[q] holds (c'-part, m-free) where m = 8d + 2h + s and
                # c' = n_ref % 512.
                # matmuls write contiguous 128-col PSUM blocks (strided
                # matmul PSUM writes miscompile); the d/e interleave of the
                # torch scramble happens in the permuted eviction copy.
                for q in range(CCH):
                    xs_ps = ps_big.tile([128, N], FP, tag="big", name=f"xsps{q}")
                    for s in range(2):
                        k_ref = 4 * s + q
                        for h in range(HEADS):
                            j = 2 * h + s
                            pe.matmul(xs_ps[:, ts(j, 128)],
                                      at_sbs[h][:, ts(k_ref, 128)],
                                      vpt_sbs[h][:], start=True, stop=True,
                                      skip_group_check=True)
                    # columns are permuted from torch-scramble row order m =
                    # 8d+e to the y-row order n'(m) = 256*(d%4) + 32e + d//4,
                    # so OUTA lhsT chunks line up with n'-ordered y rows
                    dve.tensor_copy(
                        xsa[q][:].rearrange("p (dl e dh) -> p dl e dh",
                                            dl=4, e=8),
                        xs_ps[:].rearrange("p (e dh dl) -> p dl e dh",
                                           e=8, dh=32))

                # ---- OUTA (n', 512) = bias + [x_sa@out_w^T | x_ca@out2_w^T]
                for m in range(NCH):
                    o_ps = ps_med.tile([128, C], FP, tag="sm", name=f"ops{m}")
                    pe.matmul(o_ps[:], ones_row[0:1, :], cst["brf"][:],
                              start=True, stop=False, skip_group_check=True)
                    for q in range(CCH):
                        pe.matmul(o_ps[:, 0:C // 2], xsa[q][:, ts(m, 128)],
                                  cst["owt"][q][:], start=False, stop=(q == CCH - 1),
                                  skip_group_check=True)
                    for h in range(HEADS):
                        pe.matmul(o_ps[:, C // 2:C], xca[h][:, ts(m, 128)],
                                  cst["owt2"][h][:], start=False, stop=(h == HEADS - 1),
                                  skip_group_check=True)
                    if m % 2 == 0:
                        act.activation(outa[m][:], o_ps[:], AF.Identity)
                    else:
                        dve.tensor_copy(outa[m][:], o_ps[:])

        # ======================= phase B: LFE =======================
        if "B" in phases:
          with ExitStack() as pb:
            g_pool = pb.enter_context(tc.tile_pool(name=f"g{label}", bufs=1))
            ps_big2 = pools["big"]
            conv_pool = pb.enter_context(tc.tile_pool(name=f"conv{label}", bufs=2))
            g_tiles = []

            NPT = len(PE_TAPS)
            dg4 = cst["dgP"][:].rearrange("p (o t j) -> p o t j", t=NPT, j=128)
            for oc in range(OCH):
                # T1 = fc1 chunk; evict with bias into zero-padded bf16 tile
                t1_ps = ps_big2.tile([128, N], FP, tag="big", name=f"t1ps{oc}")
                for j2 in range(2):
                    for i in range(CCH):
                        pe.matmul(t1_ps[:, ts(j2, 512)],
                                  cst["fc1T"][i][:, ts(oc, 128)],
                                  xcb[i][:, ts(j2, 512)],
                                  start=(i == 0), stop=(i == CCH - 1))
                t_sb = conv_pool.tile([128, N + 96], BF, tag="t_sb")
                dve.memset(t_sb[:, 0:48], 0.0)
                dve.memset(t_sb[:, N + 48:N + 96], 0.0)
                act.activation(t_sb[:, 48:N + 48], t1_ps[:],
                               AF.Identity, bias=cst["fb1"][:, oc:oc + 1])
                tflat = t_sb[:]
                # (p, w, h) view of the unpadded region for wrap fixups
                tvv = t_sb[:, 48:N + 48].rearrange("p (w h) -> p w h", h=H_)

                def tap_src(kh, kw):
                    s = (kh - 1) + 32 * (kw - 1)
                    return tflat[:, 48 + s:48 + s + N]

                # PE partial: |PE_TAPS| diag matmuls accumulated in PSUM
                cv_ps = ps_big2.tile([128, N], FP, tag="big", name=f"cv{oc}")
                for t, (kh, kw) in enumerate(PE_TAPS):
                    srcw = tap_src(kh, kw)
                    for u in (0, 512):
                        pe.matmul(cv_ps[:, u:u + 512], dg4[:, oc, t, :],
                                  srcw[:, u:u + 512],
                                  start=(t == 0), stop=(t == NPT - 1),
                                  skip_group_check=True)

                # center tap + PSUM merge in one stt, then DVE/GPS taps
                acc = conv_pool.tile([128, N], BF, tag="acc")
                accv = acc[:].rearrange("p (w h) -> p w h", h=H_)
                kh, kw = CENTER_TAP
                dve.scalar_tensor_tensor(
                    acc[:], tap_src(kh, kw),
                    cst["dww"][:, oc * 9 + 3 * kh + kw:oc * 9 + 3 * kh + kw + 1],
                    cv_ps[:], op0=ALU.mult, op1=ALU.add)
                for kh, kw in DVE_TAPS + GPS_TAPS:
                    eng = gps if (kh, kw) in GPS_TAPS else dve
                    wcol = cst["dww"][:, oc * 9 + 3 * kh + kw:oc * 9 + 3 * kh + kw + 1]
                    eng.scalar_tensor_tensor(acc[:], tap_src(kh, kw), wcol,
                                             acc[:], op0=ALU.mult, op1=ALU.add)
                # h-wrap column fixups (oh != 0 taps): subtract the wrapped
                # contribution and add the correct zero-pad (i.e. just remove).
                for kh, kw in DVE_FIX + GPS_FIX:
                    t_i = 3 * kh + kw
                    oh, ow = kh - 1, kw - 1
                    # At the h-boundary column (h=31 for oh=+1, h=0 for oh=-1)
                    # the flat read n'+s = 32(w+ow) + (h+oh) wraps into the
                    # neighbouring w column: w' = w+ow+1, h'=0 (oh=+1) or
                    # w' = w+ow-1, h'=31 (oh=-1). The intended value is the
                    # SAME-pad zero, so subtract the wrongly-added term for
                    # every site whose wrapped read landed in-tile
                    # (w' in [0,32); out-of-tile reads hit the zero pad).
                    hb = 31 if oh == 1 else 0
                    hp = 0 if oh == 1 else 31
                    wp_off = ow + (1 if oh == 1 else -1)
                    w0 = max(0, -wp_off)
                    w1 = min(W_, W_ - wp_off)
                    nwc = cst["ndww"][:, oc * 9 + t_i:oc * 9 + t_i + 1]
                    feng = gps if (kh, kw) in GPS_FIX else dve
                    feng.scalar_tensor_tensor(
                        accv[:, w0:w1, hb:hb + 1],
                        tvv[:, w0 + wp_off:w1 + wp_off, hp:hp + 1], nwc,
                        accv[:, w0:w1, hb:hb + 1],
                        op0=ALU.mult, op1=ALU.add)

                g_sb = g_pool.tile([128, N], BF, tag=f"g{oc}")
                if sim_gelu:
                    gt = conv_pool.tile([128, N], FP, tag="gt")
                    act.activation(gt[:], acc[:], AF.Identity,
                                   bias=cst["dwb"][:, oc:oc + 1])
                    sg = conv_pool.tile([128, N], FP, tag="sg")
                    act.activation(sg[:], gt[:], AF.Sigmoid, scale=1.702)
                    dve.tensor_mul(g_sb[:], gt[:], sg[:])
                else:
                    act.activation(g_sb[:], acc[:], AF.Gelu,
                                   bias=cst["dwb"][:, oc:oc + 1])
                g_tiles.append(g_sb)

            # ---- fc2 + OUTA -> y
            with ExitStack() as pf2:
                fin_pool = pf2.enter_context(tc.tile_pool(name=f"fin{label}", bufs=2))
                ps_fc2 = pools["sm"]
                ydst = aps["y"][b].rearrange("w h c -> (w h) c")  # (N, C)
                for m in range(NCH):
                    f_ps = ps_fc2.tile([128, C], FP, tag="sm", name=f"fps{m}")
                    for oc in range(OCH):
                        pe.matmul(f_ps[:], g_tiles[oc][:, ts(m, 128)],
                                  cst["fc2T"][oc][:],
                                  start=(oc == 0), stop=(oc == OCH - 1))
                    fin = fin_pool.tile([128, C], FP, tag="fin")
                    dve.tensor_add(fin[:], f_ps[:], outa[m][:])
                    nc.sync.dma_start(ydst[m * 128:(m + 1) * 128], fin[:])


_BUILD_CACHE = {}


def _get_nc():
    if "nc" not in _BUILD_CACHE:
        _BUILD_CACHE["nc"] = build()
    return _BUILD_CACHE["nc"]


def kernel(**inputs):
    from concourse.bass_utils import run_bass_kernel_spmd

    def f32(a):
        return np.ascontiguousarray(np.asarray(a, dtype=np.float32))

    x = f32(inputs["x"])
    assert x.shape == (B, W_, H_, C), x.shape
    common = {k: f32(inputs[k]) for k in
              ("qkvv_w", "E_w", "E_b", "F_w", "F_b", "temp", "temp2",
               "out_w", "out_b", "out2_w", "out2_b",
               "fc1_w", "fc1_b", "dw_w", "dw_b", "fc2_w", "fc2_b")}

    nc = _get_nc()
    in_maps = []
    for c in range(NCORES):
        m = dict(common)
        m["x"] = np.ascontiguousarray(x[c * BL:(c + 1) * BL])
        in_maps.append(m)

    res = run_bass_kernel_spmd(nc, in_maps, list(range(NCORES)))
    out = np.concatenate([res.results[c]["y"] for c in range(NCORES)], axis=0)
    return out.astype(np.float32)


# revision 62
# speedup vs baseline: 2.1036x; 2.1036x over previous
"""Trainium2 Bass kernel for nn_DSFE (dual attention + LFE depthwise-conv block).

Sharding: pure data-parallel over batch B=16 across 8 NeuronCores (2 batches
per core), no collectives.

v2 design (vs v0 baseline):
  * bf16 everywhere on the PE (1 cycle/row at any width, 1024-wide moving
    operands, 1c/row transposes); PSUM accumulation stays fp32.  Empirical
    bf16-everywhere error vs the fp32 reference: 3.6e-3 absmax-rel (budget
    2e-2).  fp8 measured 3-6e-2 -> rejected.
  * spatial index n' = 32w + h (x's native memory order) everywhere: x loads,
    XC transposes, y stores are all contiguous; E/F weights are loaded through
    a (h w)->(w h) permuted AP; the one place the torch reshape semantically
    mixes spatial order into channels (x_sa scramble) reads at_sb through an
    n_ref-permuted AP when forming x_sa lhsT chunks.
  * all transposed weights (qkvvT/fc1T/fc2T/owt/owt2/ewt/fwt) are cast to
    bf16 and kept resident in SBUF - no DRAM scratch roundtrip, no per-batch
    weight reloads.
  * 3x3 depthwise conv: 6 taps as bf16 diag matmuls on the PE (PSUM), the
    center tap + PSUM merge as one DVE stt, 2 more taps on DVE, h-wrap
    column fixups on DVE.  (GPSIMD is unusable here: no float stt/TT ops in
    this ISA, and its software memsets cost ~100s of us each.)
  * one shared set of PSUM pools across all phases/batches (scoped pools
    serialize phase overlap through bank-reuse barriers); per-head small
    PSUM tensors live as slot views of one rotating (128,512) bank.
  * PSUM evictions split between ACT (activation Identity, with fused
    bias/scale/cast) and DVE to balance engine load; norms for all heads
    hoisted out of the head loop (native squares + ones-matmuls).

HW lowering constraints found the hard way: matmul moving operands max 512
free elements; matmul operands must be single-free-dim APs; strided matmul
PSUM writes miscompile; partition-offset reads must be 32-aligned.

TimelineSim: 503 us/core vs 692 us for the v0 fp32r baseline (the axon
round-trip jitter, +-1.5 ms/call, prevents a clean HW wall-clock number).
"""

from contextlib import ExitStack

import numpy as np

import concourse.bass as bass
import concourse.mybir as mybir
import concourse.tile as tile
from concourse.masks import make_identity

FP = mybir.dt.float32
BF = mybir.dt.bfloat16
AF = mybir.ActivationFunctionType
ALU = mybir.AluOpType
ts = bass.ts

B, W_, H_, C = 16, 32, 32, 512
HEADS, PP, HID = 4, 16, 2048
N = H_ * W_            # 1024
D = C // HEADS         # 128
NCORES = 8
BL = B // NCORES       # 2 batches per core
NCH = N // 128         # 8
CCH = C // 128         # 4
OCH = HID // 128       # 16

NORM_EPS = 1e-12

# depthwise 3x3 taps (kh, kw); spatial n' = 32w + h so shift = oh + 32*ow.
# PE_TAPS run as bf16 diag matmuls accumulated in PSUM; the first DVE op
# merges the PSUM partial with the center tap (stt in1=psum); remaining taps
# split DVE/GPSIMD.  oh != 0 taps need an h-wrap column fixup (split DVE/GPS).
CENTER_TAP = (1, 1)
PE_TAPS = [(0, 0), (0, 2), (2, 0), (2, 2), (1, 2), (2, 1)]
DVE_TAPS = [(0, 1), (1, 0)]
GPS_TAPS = []
DVE_FIX = [(0, 0), (0, 1), (0, 2), (2, 2), (2, 0), (2, 1)]
GPS_FIX = []


def split_multi_waits(nc):
    """This environment's walrus build encodes at most ONE sync wait per
    instruction; re-host excess waits onto same-engine NoOps just before."""
    n_split = 0
    for f in nc.m.functions:
        for blk in f.blocks:
            new = []
            changed = False
            for inst in blk.instructions:
                si = inst.sync_info
                waits = list(si.on_wait) if si and si.on_wait else []
                if len(waits) > 1:
                    n_split += 1
                    changed = True
                    for w in waits[:-1]:
                        new.append(mybir.InstNoOp(
                            name=nc.get_next_instruction_name(),
                            sync_info=mybir.SyncInfo(on_wait=[w], on_update=[]),
                            bass_nofuse=True,
                            engine=inst.engine,
                        ))
                    inst.sync_info = mybir.SyncInfo(
                        on_wait=[waits[-1]],
                        on_update=list(si.on_update) if si.on_update else [],
                    )
                new.append(inst)
            if changed:
                blk.instructions = new
    return n_split


def build(split_waits=True, sim_gelu=False, loop=1, phases="AB"):
    """Build the per-core Bass module (SPMD: identical program on 8 cores)."""
    nc = bass.Bass("TRN2", target_bir_lowering=False, debug=False)

    def din(name, shape):
        return nc.dram_tensor(name, list(shape), FP, kind="ExternalInput").ap()

    aps = dict(
        x=din("x", (BL, W_, H_, C)),
        qkvv_w=din("qkvv_w", (4 * C, C)),
        e_w=din("E_w", (PP, N)),
        e_b=din("E_b", (PP,)),
        f_w=din("F_w", (PP, N)),
        f_b=din("F_b", (PP,)),
        temp=din("temp", (HEADS, 1, 1)),
        temp2=din("temp2", (HEADS, 1, 1)),
        out_w=din("out_w", (C // 2, C)),
        out_b=din("out_b", (C // 2,)),
        out2_w=din("out2_w", (C // 2, C)),
        out2_b=din("out2_b", (C // 2,)),
        fc1_w=din("fc1_w", (HID, C)),
        fc1_b=din("fc1_b", (HID,)),
        dw_w=din("dw_w", (HID, 1, 3, 3)),
        dw_b=din("dw_b", (HID,)),
        fc2_w=din("fc2_w", (C, HID)),
        fc2_b=din("fc2_b", (C,)),
        y=nc.dram_tensor("y", [BL, W_, H_, C], FP, kind="ExternalOutput").ap(),
    )

    with tile.TileContext(nc) as tc:
        _emit(nc, tc, aps, sim_gelu, loop, phases)

    if split_waits:
        split_multi_waits(nc)
    return nc


def _transpose_weight_bf(nc, tc, pools, identb, nat_ap, dst, label):
    """Transpose a natural (R, K) fp32 DRAM weight into K-major bf16 SBUF
    tiles dst[K//128] of shape (128, R).

    Casting gpsimd DMAs load fp32 DRAM directly into bf16 SBUF (one per
    8-row-chunk group, so transposes start after the first chunk lands);
    bf16 PE transposes batched 8 row-chunks per (128,1024) PSUM tile, one
    DVE evict each."""
    act = nc.scalar
    dve = nc.vector
    pe = nc.tensor
    R, K = nat_ap.shape
    RC, KC = R // 128, K // 128
    G = min(8, RC)  # row-chunks per PSUM eviction batch
    with ExitStack() as es:
        nat_pool = es.enter_context(tc.tile_pool(name=f"twn_{label}", bufs=1))
        nat = nat_pool.tile([128, RC, K], FP, tag="nat", name=f"nat_{label}")
        natb = nat_pool.tile([128, RC, K], BF, tag="natb", name=f"natb_{label}")
        natsrc = nat_ap.rearrange("(g p) k -> p g k", p=128)
        for g in range((RC + G - 1) // G):
            g0, g1 = g * G, min((g + 1) * G, RC)
            nc.sync.dma_start(nat[:, g0:g1, :], natsrc[:, g0:g1, :])
        for rj in range(RC):
            act.activation(natb[:, rj, :], nat[:, rj, :], AF.Identity)
        for g in range((RC + G - 1) // G):
            rjs = list(range(g * G, min((g + 1) * G, RC)))
            for ci in range(KC):
                ps = pools["tr"].tile([128, 1024], BF, tag="tr",
                                      name=f"tw_{label}{g}_{ci}")
                for j, rj in enumerate(rjs):
                    pe.transpose(ps[:, ts(j, 128)], natb[:, rj, ts(ci, 128)],
                                 identb[:])
                dve.tensor_copy(
                    dst[ci][:, g * G * 128:g * G * 128 + len(rjs) * 128],
                    ps[:, 0:len(rjs) * 128])


def _emit(nc, tc, aps, sim_gelu=False, loop=1, phases="AB"):
    act = nc.scalar
    dve = nc.vector
    gps = nc.gpsimd
    pe = nc.tensor

    with ExitStack() as es:
        const = es.enter_context(tc.tile_pool(name="const", bufs=1))

        # one shared set of PSUM pools for the whole program (8 banks total):
        # scoped per-phase pools would serialize phase/batch overlap through
        # bank reuse barriers.
        ps_big = es.enter_context(tc.tile_pool(name="ps_big", bufs=2, space="PSUM"))
        ps_tr = es.enter_context(tc.tile_pool(name="ps_tr", bufs=2, space="PSUM"))
        ps_sm = es.enter_context(tc.tile_pool(name="ps_sm", bufs=2, space="PSUM"))
        pools = dict(big=ps_big, tr=ps_tr, sm=ps_sm)

        identb = const.tile([128, 128], BF)
        make_identity(nc, identb)
        ident32 = const.tile([128, 128], FP)
        make_identity(nc, ident32)
        ones_row = const.tile([1, 128], BF)
        dve.memset(ones_row[:], 1.0)
        ones_col = const.tile([128, 1], BF)
        dve.memset(ones_col[:], 1.0)

        # resident bf16 transposed weights
        qkvvT = [const.tile([128, 4 * C], BF, tag=f"qkvvT{i}", name=f"qkvvT{i}") for i in range(CCH)]
        _transpose_weight_bf(nc, tc, pools, identb, aps["qkvv_w"], qkvvT, "qkvv")
        fc1T = [const.tile([128, HID], BF, tag=f"fc1T{i}", name=f"fc1T{i}") for i in range(CCH)]
        _transpose_weight_bf(nc, tc, pools, identb, aps["fc1_w"], fc1T, "fc1")
        fc2T = [const.tile([128, C], BF, tag=f"fc2T{o}", name=f"fc2T{o}") for o in range(OCH)]
        _transpose_weight_bf(nc, tc, pools, identb, aps["fc2_w"], fc2T, "fc2")
        owt = [const.tile([128, C // 2], BF, tag=f"owt{q}", name=f"owt{q}") for q in range(CCH)]
        _transpose_weight_bf(nc, tc, pools, identb, aps["out_w"], owt, "outw")
        owt2 = [const.tile([128, C // 2], BF, tag=f"owt2{q}", name=f"owt2{q}") for q in range(CCH)]
        _transpose_weight_bf(nc, tc, pools, identb, aps["out2_w"], owt2, "outw2")

        # temp/temp2 broadcast to all 128 partitions: (128, HEADS) fp32
        tcol = const.tile([128, HEADS], FP)
        nc.sync.dma_start(
            tcol[:], aps["temp"].rearrange("h a b -> (a b) h").to_broadcast((128, HEADS)))
        t2col = const.tile([128, HEADS], FP)
        nc.sync.dma_start(
            t2col[:], aps["temp2"].rearrange("h a b -> (a b) h").to_broadcast((128, HEADS)))

        # E^T / F^T as (n'-part r, n'-chunk k, p) bf16; n' order via (h w)->(w h)
        ewt = const.tile([128, NCH, PP], BF)
        fwt = const.tile([128, NCH, PP], BF)
        with ExitStack() as ef:
            ef_pool = ef.enter_context(tc.tile_pool(name="ef_nat", bufs=2))
            for nm, dst in (("e_w", ewt), ("f_w", fwt)):
                natf = ef_pool.tile([PP, N], FP, tag="natf", name=f"{nm}_natf")
                nc.sync.dma_start(natf[:], aps[nm][:])
                natb = ef_pool.tile([PP, N], BF, tag="natb", name=f"{nm}_natb")
                act.activation(natb[:], natf[:], AF.Identity)
                # reorder columns to n' = 32w + h with a permuted DVE copy
                # (matmul operands must have a single free dimension)
                nprm = ef_pool.tile([PP, N], BF, tag="nprm", name=f"{nm}_nprm")
                dve.tensor_copy(nprm[:].rearrange("p (w h) -> p w h", h=H_),
                                natb[:].rearrange("p (h w) -> p w h", w=W_))
                for k in range(NCH):
                    ps = pools["sm"].tile([128, 512], FP, tag="sm",
                                          name=f"efps_{nm}{k}")
                    psv = ps[:, 0:PP // 2].bitcast(BF)
                    pe.transpose(psv, nprm[:, ts(k, 128)], identb[0:PP, 0:PP])
                    dve.tensor_copy(dst[:, k, :], psv)
        eb_row = const.tile([1, PP], BF)
        e_b32 = const.tile([1, PP], FP)
        nc.sync.dma_start(e_b32[:], aps["e_b"].unsqueeze(0))
        dve.tensor_copy(eb_row[:], e_b32[:])
        fb_row = const.tile([1, PP], BF)
        f_b32 = const.tile([1, PP], FP)
        nc.sync.dma_start(f_b32[:], aps["f_b"].unsqueeze(0))
        dve.tensor_copy(fb_row[:], f_b32[:])

        # free-dim bias row for OUTA: concat(out_b, out2_b) + fc2_b, bf16
        br_cat = const.tile([1, C], FP)
        nc.sync.dma_start(br_cat[:, 0:C // 2], aps["out_b"].unsqueeze(0))
        nc.sync.dma_start(br_cat[:, C // 2:C], aps["out2_b"].unsqueeze(0))
        fcb_row = const.tile([1, C], FP)
        nc.sync.dma_start(fcb_row[:], aps["fc2_b"].unsqueeze(0))
        brf = const.tile([1, C], BF)
        dve.tensor_add(brf[:], br_cat[:], fcb_row[:])

        # per-partition (o) bias/weight columns for the LFE branch (fp32)
        fb1 = const.tile([128, OCH], FP)
        nc.sync.dma_start(fb1[:], aps["fc1_b"].rearrange("(k r) -> r k", r=128))
        dwb = const.tile([128, OCH], FP)
        nc.sync.dma_start(dwb[:], aps["dw_b"].rearrange("(k r) -> r k", r=128))
        dww = const.tile([128, OCH * 9], FP)
        for oc in range(OCH):
            nc.sync.dma_start(
                dww[:, oc * 9:(oc + 1) * 9],
                aps["dw_w"][oc * 128:(oc + 1) * 128].rearrange("r a kh kw -> r (a kh kw)"))
        ndww = const.tile([128, OCH * 9], FP)
        dve.tensor_scalar_mul(ndww[:], dww[:], -1.0)

        # bf16 diag(w_tap) tiles for the PE conv taps: (128, OCH, |PE_TAPS|, 128)
        NPT = len(PE_TAPS)
        dgP = const.tile([128, OCH * NPT * 128], BF)
        dg4 = dgP[:].rearrange("p (o t j) -> p o t j", t=NPT, j=128)
        for oc in range(OCH):
            wsel = const.tile([128, NPT], FP, tag="wsel", name=f"wsel{oc}")
            for t, (kh, kw) in enumerate(PE_TAPS):
                dve.tensor_copy(wsel[:, t:t + 1],
                                dww[:, oc * 9 + 3 * kh + kw:oc * 9 + 3 * kh + kw + 1])
            dve.tensor_mul(dg4[:, oc, :, :],
                           identb[:].unsqueeze(1).to_broadcast((128, NPT, 128)),
                           wsel[:].unsqueeze(2).to_broadcast((128, NPT, 128)))

        cst = dict(identb=identb, ident32=ident32, ones_row=ones_row,
                   ones_col=ones_col,
                   ndww=ndww, dgP=dgP, tcol=tcol, t2col=t2col,
                   ewt=ewt, fwt=fwt, eb_row=eb_row, fb_row=fb_row, brf=brf,
                   fb1=fb1, dwb=dwb, dww=dww, owt=owt, owt2=owt2,
                   qkvvT=qkvvT, fc1T=fc1T, fc2T=fc2T)

        for rep in range(loop):
            for b in range(BL):
                _emit_batch(nc, tc, b, aps, cst, pools, sim_gelu,
                            label=f"{rep}_{b}", phases=phases)


def _emit_batch(nc, tc, b, aps, cst, pools, sim_gelu=False, label=None, phases="AB"):
    if label is None:
        label = str(b)
    act = nc.scalar
    dve = nc.vector
    gps = nc.gpsimd
    pe = nc.tensor
    identb = cst["identb"]
    ones_row = cst["ones_row"]

    with ExitStack() as bs:
        xc_pool = bs.enter_context(tc.tile_pool(name=f"xc{label}", bufs=1))
        outa_pool = bs.enter_context(tc.tile_pool(name=f"outa{label}", bufs=1))

        # ---- XC (c, n') bf16, n' = 32w + h == x's native row order.
        xcb = [xc_pool.tile([128, N], BF, tag=f"xc{i}", name=f"xc{i}")
               for i in range(CCH)]
        with ExitStack() as xl:
            xn_pool = xl.enter_context(tc.tile_pool(name=f"xn{label}", bufs=1))
            xn = xn_pool.tile([128, NCH, C], FP, tag="xn", name=f"xn{label}")
            xnb = xn_pool.tile([128, NCH, C], BF, tag="xnb", name=f"xnb{label}")
            xsrc = (aps["x"][b].rearrange("w h c -> (w h) c")
                    .rearrange("(g p) c -> p g c", p=128))
            for g in range(2):
                nc.sync.dma_start(xn[:, 4 * g:4 * (g + 1), :],
                                  xsrc[:, 4 * g:4 * (g + 1), :])
            for kk in range(NCH):
                act.activation(xnb[:, kk, :], xn[:, kk, :], AF.Identity)
            for i in range(CCH):
                ps = pools["tr"].tile([128, 1024], BF, tag="tr",
                                      name=f"xct{i}")
                for kk in range(NCH):
                    pe.transpose(ps[:, ts(kk, 128)], xnb[:, kk, ts(i, 128)],
                                 identb[:])
                dve.tensor_copy(xcb[i][:], ps[:])

        outa = [outa_pool.tile([128, C], FP, tag=f"outa{m}", name=f"outa{m}") for m in range(NCH)]
        if phases != "AB":
            for m in range(NCH):
                dve.memset(outa[m][:], 0.0)
        if "B" not in phases:
            ydst0 = aps["y"][b].rearrange("w h c -> (w h) c")
            nc.sync.dma_start(ydst0[0:128], outa[0][:])

        # ======================= phase A: attention =======================
        if "A" in phases:
          with ExitStack() as pa:
            qkvv_pool = pa.enter_context(tc.tile_pool(name=f"qkvv{label}", bufs=1))
            ps_big = pools["big"]
            ps_t = pools["tr"]
            # one (128,512) fp32 bank per head rotation hosts every small
            # PSUM tensor as a slot view (PSUM tiles round up to full banks)
            ps_med = pools["sm"]

            # ---- QKVV (n', 4C) bf16, plus elementwise squares of the q|k
            # half for the hoisted norm computation
            qkvv = [qkvv_pool.tile([128, 4 * C], BF, tag=f"qkvv{k}", name=f"qkvv{k}") for k in range(NCH)]
            sqq = qkvv_pool.tile([128, NCH, 1024], BF, tag="sqq", name=f"sqq{label}")
            for k in range(NCH):
                for half in range(2):
                    ps = ps_big.tile([128, 1024], FP, tag="big", name=f"qkps{k}_{half}")
                    for j2 in range(2):
                        for i in range(CCH):
                            pe.matmul(ps[:, ts(j2, 512)], xcb[i][:, ts(k, 128)],
                                      cst["qkvvT"][i][:, half * 1024 + j2 * 512:half * 1024 + (j2 + 1) * 512],
                                      start=(i == 0), stop=(i == CCH - 1))
                    act.activation(qkvv[k][:, ts(half, 1024)], ps[:], AF.Identity)
                act.activation(sqq[:, k, :], qkvv[k][:, 0:1024], AF.Square)

            with ExitStack() as ph:
                at_pool = pa.enter_context(tc.tile_pool(name=f"at{label}", bufs=2))
                hd_pool = pa.enter_context(tc.tile_pool(name=f"hd{label}", bufs=1))
                xca_pool = pa.enter_context(tc.tile_pool(name=f"xca{label}", bufs=1))
                xsa_pool = pa.enter_context(tc.tile_pool(name=f"xsa{label}", bufs=1))

                xsa = [xsa_pool.tile([128, N], BF, tag=f"xsa{q}", name=f"xsa{q}") for q in range(CCH)]
                xca = []
                at_sbs = []
                vpt_sbs = []

                # ---- hoisted norms for all heads: ssq[d, c8] over n' via
                # ones-matmuls (c8 = 4 q-heads then 4 k-heads), then one
                # sqrt/recip, and the per-head CA column scale as one
                # (128, 4*128) outer product.  norms are O(sqrt(N)) here, so
                # the reference's 1e-12 clamp is a no-op.
                hsm = ps_med.tile([128, 512], FP, tag="sm", name=f"hsm{label}")
                nrm_ps = hsm[:, 0:8]
                for c8 in range(8):
                    for k in range(NCH):
                        pe.matmul(nrm_ps[:, c8:c8 + 1],
                                  sqq[:, k, ts(c8, 128)], cst["ones_col"][:],
                                  start=(k == 0), stop=(k == NCH - 1),
                                  skip_group_check=True)
                nrm8 = at_pool.tile([128, 8], FP, tag="nrm8")
                act.activation(nrm8[:], nrm_ps, AF.Sqrt)
                inv8 = at_pool.tile([128, 8], FP, tag="inv8")
                dve.reciprocal(inv8[:], nrm8[:])
                invq_t4 = at_pool.tile([128, HEADS], FP, tag="invq_t4")
                dve.tensor_mul(invq_t4[:], inv8[:, 0:4], cst["tcol"][:])
                invkb4 = at_pool.tile([128, HEADS], BF, tag="invkb4")
                dve.tensor_copy(invkb4[:], inv8[:, 4:8])
                # per-head (128,1)->(1,128) transposes, all on partition 0
                # (partition-offset reads must be 32-aligned in walrus)
                ikr_row = at_pool.tile([1, 4 * 128], BF, tag="ikr_row")
                for h in range(HEADS):
                    ikr_ps_h = hsm[0:1, 16 + 64 * h:80 + 64 * h].bitcast(BF)
                    pe.transpose(ikr_ps_h, invkb4[:, h:h + 1], identb[:])
                    dve.tensor_copy(ikr_row[:, ts(h, 128)], ikr_ps_h)
                csm = ps_med.tile([128, 512], FP, tag="sm", name=f"csm{label}")
                pe.matmul(csm[:], ones_row[:], ikr_row[:], start=True, stop=True)
                colsc_sb = at_pool.tile([128, 512], BF, tag="colsc_sb")
                dve.tensor_copy(colsc_sb[:], csm[:])

                for h in range(HEADS):
                    qc = h * 128            # q columns in QKVV
                    kc = C + h * 128        # k columns
                    vc = 2 * C + h * 128    # v_ca columns
                    sc = 3 * C + h * 128    # v_sa columns

                    # one bank hosts all small PSUM tensors for this head:
                    #   [0:128) s_ps / (reused) [0:64) et_ps(bf16)
                    #   [128:256) a_ps   [256:272) kp   [272:288) vp
                    #   [288:352) p0:16 vpt(bf16)
                    sm = ps_med.tile([128, 512], FP, tag="sm", name=f"sm{h}")
                    s_ps = sm[:, 0:128]
                    et_ps = sm[:, 0:64].bitcast(BF)
                    a_ps = sm[:, 128:256]
                    kp_ps = sm[:, 256:272]
                    vp_ps = sm[:, 272:288]
                    vpt_ps = sm[0:16, 288:352].bitcast(BF)

                    # ---- q^T (d, n') -> qn_t
                    qT_ps = ps_t.tile([128, N], BF, tag="tr", name=f"qT{h}")
                    for k in range(NCH):
                        pe.transpose(qT_ps[:, ts(k, 128)],
                                     qkvv[k][:, qc:qc + 128], identb[:])
                    qn_t = at_pool.tile([128, N], BF, tag="qn_t")
                    act.activation(qn_t[:], qT_ps[:], AF.Identity,
                                   scale=inv8[:, h:h + 1])

                    # ---- CA scores S0 = q @ k^T (d x d)
                    for k in range(NCH):
                        pe.matmul(s_ps, qkvv[k][:, qc:qc + 128],
                                  qkvv[k][:, kc:kc + 128],
                                  start=(k == 0), stop=(k == NCH - 1))
                    s_sb = at_pool.tile([128, 128], FP, tag="s_sb")
                    dve.scalar_tensor_tensor(s_sb[:], s_ps, invq_t4[:, h:h + 1],
                                             colsc_sb[:, ts(h, 128)],
                                             op0=ALU.mult, op1=ALU.mult)

                    # ---- CA row softmax, no max-subtraction: logits are
                    # temp * cos-similarity, |logit| <= |temp| (bounded) so
                    # exp cannot overflow.  1/sum folded into the x_ca evict.
                    e_sb = at_pool.tile([128, 128], BF, tag="e_sb")
                    sume = at_pool.tile([128, 1], FP, tag="sume")
                    act.activation(e_sb[:], s_sb[:], AF.Exp, accum_out=sume[:])
                    rex = at_pool.tile([128, 1], FP, tag="rex")
                    dve.reciprocal(rex[:], sume[:])

                    pe.transpose(et_ps, e_sb[:], identb[:])
                    et_sb = at_pool.tile([128, 128], BF, tag="et_sb")
                    dve.tensor_copy(et_sb[:], et_ps)

                    # ---- v_ca^T (e, n')
                    vt_ps = ps_t.tile([128, N], BF, tag="tr", name=f"vT{h}")
                    for k in range(NCH):
                        pe.transpose(vt_ps[:, ts(k, 128)],
                                     qkvv[k][:, vc:vc + 128], identb[:])
                    vt_sb = at_pool.tile([128, N], BF, tag="vt_sb")
                    act.activation(vt_sb[:], vt_ps[:], AF.Identity)

                    # ---- x_ca (d, n') = (1/sum) * exp(S)^T.T @ v_ca^T
                    xca_ps = ps_big.tile([128, N], FP, tag="big", name=f"xcaps{h}")
                    for j2 in range(2):
                        pe.matmul(xca_ps[:, ts(j2, 512)], et_sb[:],
                                  vt_sb[:, ts(j2, 512)], start=True, stop=True)
                    xca_h = xca_pool.tile([128, N], BF, tag=f"xca{h}")
                    act.activation(xca_h[:], xca_ps[:], AF.Identity, scale=rex[:])
                    xca.append(xca_h)

                    # ---- SA: k_proj / v_proj (d, p), bias first
                    pe.matmul(kp_ps, ones_row[:], cst["eb_row"][:],
                              start=True, stop=False)
                    for k in range(NCH):
                        pe.matmul(kp_ps, qkvv[k][:, kc:kc + 128],
                                  cst["ewt"][:, k, :], start=False, stop=(k == NCH - 1))
                    kp_sb = at_pool.tile([128, PP], BF, tag="kp_sb")
                    dve.tensor_copy(kp_sb[:], kp_ps)

                    pe.matmul(vp_ps, ones_row[:], cst["fb_row"][:],
                              start=True, stop=False)
                    for k in range(NCH):
                        pe.matmul(vp_ps, qkvv[k][:, sc:sc + 128],
                                  cst["fwt"][:, k, :], start=False, stop=(k == NCH - 1))
                    vp_sb = at_pool.tile([128, PP], BF, tag="vp_sb")
                    dve.tensor_copy(vp_sb[:], vp_ps)
                    pe.transpose(vpt_ps, vp_sb[:], identb[:])
                    vpt_sb = hd_pool.tile([16, 128], BF, tag=f"vpt{h}", name=f"vpt{h}")
                    dve.tensor_copy(vpt_sb[:], vpt_ps)
                    vpt_sbs.append(vpt_sb)

                    # ---- A0 (n', p) per n'-chunk, all 8 in one (128, 8, 16)
                    a3 = a_ps.rearrange("p (k s) -> p k s", s=PP)
                    for k in range(NCH):
                        pe.matmul(a3[:, k, :], qn_t[:, ts(k, 128)],
                                  kp_sb[:], start=True, stop=True)

                    # ---- segmented softmax over p, no max-subtraction:
                    # logits = temp2 * (unit qn row) . k_proj columns are
                    # bounded well inside exp range for this problem's scales
                    ez = at_pool.tile([128, 128], FP, tag="ez")
                    act.activation(ez[:], a_ps, AF.Exp, scale=cst["t2col"][:, h:h + 1])
                    ez3 = ez[:].rearrange("p (k s) -> p k s", s=PP)
                    esum = at_pool.tile([128, NCH], FP, tag="esum")
                    dve.tensor_reduce(esum[:], ez3, axis=mybir.AxisListType.X,
                                      op=ALU.add)
                    rsum = at_pool.tile([128, NCH], FP, tag="rsum")
                    dve.reciprocal(rsum[:], esum[:])
                    attn = at_pool.tile([128, 128], BF, tag="attn")
                    attn3 = attn[:].rearrange("p (k s) -> p k s", s=PP)
                    dve.tensor_mul(attn3, ez3,
                                   rsum[:].unsqueeze(2).to_broadcast((128, NCH, PP)))

                    # ---- attn^T (p, n')
                    at_full = ps_t.tile([128, N], BF, tag="tr", name=f"atps{h}")
                    for k in range(NCH):
                        pe.transpose(at_full[0:16, ts(k, 128)], attn3[:, k, :],
                                     identb[:])
                    # evict directly in n_ref order (n' -> n_ref permuted
                    # read) so x_sa lhsT chunks are flat single-free-dim APs
                    at_sb = hd_pool.tile([16, N], BF, tag=f"at{h}")
                    dve.tensor_copy(
                        at_sb[:].rearrange("p (h2 w) -> p h2 w", w=W_),
                        at_full[0:16, :].rearrange("p (w h2) -> p h2 w", w=W_))
                    at_sbs.append(at_sb)

                # ---- x_sa in the torch-scrambled layout: at_sb is already
                # in n_ref column order, so lhsT chunks are plain slices;
                # xsa[q] holds (c'-part, m-free) where m = 8d + 2h + s and
                # c' = n_ref % 512.
                # matmuls write contiguous 128-col PSUM blocks (strided
                # matmul PSUM writes miscompile); the d/e interleave of the
                # torch scramble happens in the permuted eviction copy.
                for q in range(CCH):
                    xs_ps = ps_big.tile([128, N], FP, tag="big", name=f"xsps{q}")
                    for s in range(2):
                        k_ref = 4 * s + q
                        for h in range(HEADS):
                            j = 2 * h + s
                            pe.matmul(xs_ps[:, ts(j, 128)],
                                      at_sbs[h][:, ts(k_ref, 128)],
                                      vpt_sbs[h][:], start=True, stop=True,
                                      skip_group_check=True)
                    # columns are permuted from torch-scramble row order m =
                    # 8d+e to the y-row order n'(m) = 256*(d%4) + 32e + d//4,
                    # so OUTA lhsT chunks line up with n'-ordered y rows
                    dve.tensor_copy(
                        xsa[q][:].rearrange("p (dl e dh) -> p dl e dh",
                                            dl=4, e=8),
                        xs_ps[:].rearrange("p (e dh dl) -> p dl e dh",
                                           e=8, dh=32))

                # ---- OUTA (n', 512) = bias + [x_sa@out_w^T | x_ca@out2_w^T]
                for m in range(NCH):
                    o_ps = ps_med.tile([128, C], FP, tag="sm", name=f"ops{m}")
                    pe.matmul(o_ps[:], ones_row[0:1, :], cst["brf"][:],
                              start=True, stop=False, skip_group_check=True)
                    for q in range(CCH):
                        pe.matmul(o_ps[:, 0:C // 2], xsa[q][:, ts(m, 128)],
                                  cst["owt"][q][:], start=False, stop=(q == CCH - 1),
                                  skip_group_check=True)
                    for h in range(HEADS):
                        pe.matmul(o_ps[:, C // 2:C], xca[h][:, ts(m, 128)],
                                  cst["owt2"][h][:], start=False, stop=(h == HEADS - 1),
                                  skip_group_check=True)
                    if m % 2 == 0:
                        act.activation(outa[m][:], o_ps[:], AF.Identity)
                    else:
                        dve.tensor_copy(outa[m][:], o_ps[:])

        # ======================= phase B: LFE =======================
        if "B" in phases:
          with ExitStack() as pb:
            g_pool = pb.enter_context(tc.tile_pool(name=f"g{label}", bufs=1))
            ps_big2 = pools["big"]
            conv_pool = pb.enter_context(tc.tile_pool(name=f"conv{label}", bufs=2))
            g_tiles = []

            NPT = len(PE_TAPS)
            dg4 = cst["dgP"][:].rearrange("p (o t j) -> p o t j", t=NPT, j=128)
            for oc in range(OCH):
                # T1 = fc1 chunk; evict with bias into zero-padded bf16 tile
                t1_ps = ps_big2.tile([128, N], FP, tag="big", name=f"t1ps{oc}")
                for j2 in range(2):
                    for i in range(CCH):
                        pe.matmul(t1_ps[:, ts(j2, 512)],
                                  cst["fc1T"][i][:, ts(oc, 128)],
                                  xcb[i][:, ts(j2, 512)],
                                  start=(i == 0), stop=(i == CCH - 1))
                t_sb = conv_pool.tile([128, N + 96], BF, tag="t_sb")
                dve.memset(t_sb[:, 0:48], 0.0)
                dve.memset(t_sb[:, N + 48:N + 96], 0.0)
                act.activation(t_sb[:, 48:N + 48], t1_ps[:],
                               AF.Identity, bias=cst["fb1"][:, oc:oc + 1])
                tflat = t_sb[:]
                # (p, w, h) view of the unpadded region for wrap fixups
                tvv = t_sb[:, 48:N + 48].rearrange("p (w h) -> p w h", h=H_)

                def tap_src(kh, kw):
                    s = (kh - 1) + 32 * (kw - 1)
                    return tflat[:, 48 + s:48 + s + N]

                # PE partial: |PE_TAPS| diag matmuls accumulated in PSUM
                cv_ps = ps_big2.tile([128, N], FP, tag="big", name=f"cv{oc}")
                for t, (kh, kw) in enumerate(PE_TAPS):
                    srcw = tap_src(kh, kw)
                    for u in (0, 512):
                        pe.matmul(cv_ps[:, u:u + 512], dg4[:, oc, t, :],
                                  srcw[:, u:u + 512],
                                  start=(t == 0), stop=(t == NPT - 1),
                                  skip_group_check=True)

                # center tap + PSUM merge in one stt, then DVE/GPS taps
                acc = conv_pool.tile([128, N], BF, tag="acc")
                accv = acc[:].rearrange("p (w h) -> p w h", h=H_)
                kh, kw = CENTER_TAP
                dve.scalar_tensor_tensor(
                    acc[:], tap_src(kh, kw),
                    cst["dww"][:, oc * 9 + 3 * kh + kw:oc * 9 + 3 * kh + kw + 1],
                    cv_ps[:], op0=ALU.mult, op1=ALU.add)
                for kh, kw in DVE_TAPS + GPS_TAPS:
                    eng = gps if (kh, kw) in GPS_TAPS else dve
                    wcol = cst["dww"][:, oc * 9 + 3 * kh + kw:oc * 9 + 3 * kh + kw + 1]
                    eng.scalar_tensor_tensor(acc[:], tap_src(kh, kw), wcol,
                                             acc[:], op0=ALU.mult, op1=ALU.add)
                # h-wrap column fixups (oh != 0 taps): subtract the wrapped
                # contribution and add the correct zero-pad (i.e. just remove).
                for kh, kw in DVE_FIX + GPS_FIX:
                    t_i = 3 * kh + kw
                    oh, ow = kh - 1, kw - 1
                    # At the h-boundary column (h=31 for oh=+1, h=0 for oh=-1)
                    # the flat read n'+s = 32(w+ow) + (h+oh) wraps into the
                    # neighbouring w column: w' = w+ow+1, h'=0 (oh=+1) or
                    # w' = w+ow-1, h'=31 (oh=-1). The intended value is the
                    # SAME-pad zero, so subtract the wrongly-added term for
                    # every site whose wrapped read landed in-tile
                    # (w' in [0,32); out-of-tile reads hit the zero pad).
                    hb = 31 if oh == 1 else 0
                    hp = 0 if oh == 1 else 31
                    wp_off = ow + (1 if oh == 1 else -1)
                    w0 = max(0, -wp_off)
                    w1 = min(W_, W_ - wp_off)
                    nwc = cst["ndww"][:, oc * 9 + t_i:oc * 9 + t_i + 1]
                    feng = gps if (kh, kw) in GPS_FIX else dve
                    feng.scalar_tensor_tensor(
                        accv[:, w0:w1, hb:hb + 1],
                        tvv[:, w0 + wp_off:w1 + wp_off, hp:hp + 1], nwc,
                        accv[:, w0:w1, hb:hb + 1],
                        op0=ALU.mult, op1=ALU.add)

                g_sb = g_pool.tile([128, N], BF, tag=f"g{oc}")
                if sim_gelu:
                    gt = conv_pool.tile([128, N], FP, tag="gt")
                    act.activation(gt[:], acc[:], AF.Identity,
                                   bias=cst["dwb"][:, oc:oc + 1])
                    sg = conv_pool.tile([128, N], FP, tag="sg")
                    act.activation(sg[:], gt[:], AF.Sigmoid, scale=1.702)
                    dve.tensor_mul(g_sb[:], gt[:], sg[:])
                else:
                    act.activation(g_sb[:], acc[:], AF.Gelu,
                                   bias=cst["dwb"][:, oc:oc + 1])
                g_tiles.append(g_sb)

            # ---- fc2 + OUTA -> y
            with ExitStack() as pf2:
                fin_pool = pf2.enter_context(tc.tile_pool(name=f"fin{label}", bufs=2))
                ps_fc2 = pools["sm"]
                ydst = aps["y"][b].rearrange("w h c -> (w h) c")  # (N, C)
                for m in range(NCH):
                    f_ps = ps_fc2.tile([128, C], FP, tag="sm", name=f"fps{m}")
                    for oc in range(OCH):
                        pe.matmul(f_ps[:], g_tiles[oc][:, ts(m, 128)],
                                  cst["fc2T"][oc][:],
                                  start=(oc == 0), stop=(oc == OCH - 1))
                    fin = fin_pool.tile([128, C], FP, tag="fin")
                    dve.tensor_add(fin[:], f_ps[:], outa[m][:])
                    nc.sync.dma_start(ydst[m * 128:(m + 1) * 128], fin[:])


_BUILD_CACHE = {}


def _get_nc():
    if "nc" not in _BUILD_CACHE:
        _BUILD_CACHE["nc"] = build()
    return _BUILD_CACHE["nc"]


def kernel(**inputs):
    from concourse.bass_utils import run_bass_kernel_spmd

    def f32(a):
        return np.ascontiguousarray(np.asarray(a, dtype=np.float32))

    x = f32(inputs["x"])
    assert x.shape == (B, W_, H_, C), x.shape
    common = {k: f32(inputs[k]) for k in
              ("qkvv_w", "E_w", "E_b", "F_w", "F_b", "temp", "temp2",
               "out_w", "out_b", "out2_w", "out2_b",
               "fc1_w", "fc1_b", "dw_w", "dw_b", "fc2_w", "fc2_b")}

    nc = _get_nc()
    in_maps = []
    for c in range(NCORES):
        m = dict(common)
        m["x"] = np.ascontiguousarray(x[c * BL:(c + 1) * BL])
        in_maps.append(m)

    res = run_bass_kernel_spmd(nc, in_maps, list(range(NCORES)))
    out = np.concatenate([res.results[c]["y"] for c in range(NCORES)], axis=0)
    return out.astype(np.float32)
